# revision 9
# baseline (speedup 1.0000x reference)
"""Trainium2 Bass kernel for nn_EnsembleAdaptor: batched per-member MLP.

Per ensemble member (32 total): y = relu(x @ w1.T + b1) @ w2.T + b2
with x (512, 1024), w1 (4096, 1024), b1 (4096), w2 (1024, 4096), b2 (1024).

Sharding: pure data parallel over members — 4 members per core across 8 cores.

Device algorithm per member (fp16 operands, fp32 PSUM accumulation):
  layer 1 computes hT (H on partitions): for each j-tile (32), accumulate
    8 k-tiles of  psum[j,s] += w1T_tile.T @ xT_tile,  then ScalarE
    relu(psum + b1) -> hT sbuf tile (fp16).
  layer 2 computes yT (DOUT on partitions): for each o-tile (8), accumulate
    32 k-tiles of  psum[o,s] += w2T_tile.T @ hT_tile,  then ScalarE
    identity(psum + b2) -> fp16 sbuf -> DMA out as yT.

Schedule notes (from NTFF profile analysis):
  - The PE stream floor is 2048 matmuls x 216 ns; everything else must hide
    under it.  The base schedule lost ~23 us to (a) a 10.4 us head (queue
    prologue ~7 us + serial DMA issue), (b) ~7 us of cold (1.2 GHz) matmuls
    while the HAM activity monitor warms up, (c) w1 fetch falling behind
    during member 0 because 256 KB descriptors issue serially at ~0.65 us
    each on one queue.
  - Fixes here: first critical tiles are DMA'd from the Tensor queue (its
    prologue ends ~3.6 us vs ~7 us for the others); w1 moves in 1 MB
    4-j-tile chunks on the sync queue; x and w2 ride the scalar queue's
    ring; biases and y outputs ride the gpsimd ring; y is stored fp16
    (halves output bytes; adds ~5e-4 relative error, gate is 2e-2).
"""

import contextlib
import ctypes
import os
import sys
import types

import numpy as np

import concourse.bass as bass
import concourse.tile as tile
from concourse import bacc, mybir
from concourse.bass_utils import run_bass_kernel_spmd


def _install_ntff_shim():
    """Provide antenv.axon_hooks + the ctypes NTFF profile hook when the
    image's antenv lacks them, so trace=True works under axon. Safe no-op
    on failure."""
    try:
        import antenv.axon_hooks  # noqa: F401
        return
    except ImportError:
        pass
    try:
        mod = types.ModuleType("antenv.axon_hooks")
        _state = {"hook": None}
        mod.set_axon_ntff_profile_hook = lambda h: _state.__setitem__("hook", h)
        mod.get_axon_ntff_profile_hook = lambda: _state["hook"]
        sys.modules["antenv.axon_hooks"] = mod
        import antenv
        antenv.axon_hooks = mod

        so_path = "/opt/axon/libaxon_pjrt.so"
        if not os.path.exists(so_path):
            return
        lib = ctypes.CDLL(so_path)
        if not hasattr(lib, "axon_start_nrt_profile"):
            return
        lib.axon_start_nrt_profile.argtypes = [
            ctypes.POINTER(ctypes.c_int64),
            ctypes.c_size_t,
        ]
        lib.axon_start_nrt_profile.restype = ctypes.c_int64
        lib.axon_stop_nrt_profile.argtypes = [ctypes.c_char_p]
        lib.axon_stop_nrt_profile.restype = ctypes.c_int64

        @contextlib.contextmanager
        def _hook(output_dir, device_ids):
            import jax
            jax.devices()
            if device_ids:
                ids = (ctypes.c_int64 * len(device_ids))(*device_ids)
                rc = lib.axon_start_nrt_profile(ids, len(device_ids))
            else:
                rc = lib.axon_start_nrt_profile(None, 0)
            if rc != 0:
                raise RuntimeError(f"axon_start_nrt_profile rc={rc}")
            try:
                yield
            finally:
                n = lib.axon_stop_nrt_profile(str(output_dir).encode())
                print(f"profile: {n} file(s) written to {output_dir}",
                      file=sys.stderr)

        mod.set_axon_ntff_profile_hook(_hook)
    except Exception:
        pass

B, S, DIN, H, DOUT = 32, 512, 1024, 4096, 1024
N_W1 = H * DIN
N_B1 = H
N_W2 = DOUT * H
N_B2 = DOUT

N_CORES = 8
M_PER = B // N_CORES  # members per core

DT = DIN // 128   # 8  k-tiles for layer 1
JT = H // 128     # 32 j-tiles (layer-1 outputs / layer-2 k-tiles)
OT = DOUT // 128  # 8  o-tiles for layer 2
SN = S            # 512 moving free dim

CH = 4            # j-tiles per w1 chunk
NCH = JT // CH    # 8 chunks per member

F16 = mybir.dt.float16
F32 = mybir.dt.float32
NP_F16 = np.float16

_cache = {}


def _build_nc():
    nc = bacc.Bacc("TRN2", target_bir_lowering=False, enable_partition_id=False)
    xp = nc.dram_tensor("xp", [M_PER, 128, DT * SN], F16, kind="ExternalInput")
    w1p = nc.dram_tensor("w1p", [M_PER, NCH, 128, CH * DT * 128], F16,
                         kind="ExternalInput")
    w2p = nc.dram_tensor("w2p", [M_PER, OT, 128, JT * 128], F16,
                         kind="ExternalInput")
    b1p = nc.dram_tensor("b1p", [M_PER, 128, JT], F32, kind="ExternalInput")
    b2p = nc.dram_tensor("b2p", [M_PER, 128, OT], F32, kind="ExternalInput")
    ytp = nc.dram_tensor("ytp", [M_PER, OT, 128, SN], F16, kind="ExternalOutput")

    relu = mybir.ActivationFunctionType.Relu
    ident = mybir.ActivationFunctionType.Identity

    with tile.TileContext(nc) as tc:
        with (
            tc.tile_pool(name="xpool", bufs=2) as xpool,
            tc.tile_pool(name="w1pool", bufs=6) as w1pool,
            tc.tile_pool(name="w2pool", bufs=6) as w2pool,
            tc.tile_pool(name="bpool", bufs=2) as bpool,
            tc.tile_pool(name="hpool", bufs=1) as hpool,
            tc.tile_pool(name="ypool", bufs=4) as ypool,
            tc.tile_pool(name="ps1", bufs=4, space="PSUM") as ps1pool,
            tc.tile_pool(name="ps2", bufs=4, space="PSUM") as ps2pool,
        ):
            for m in range(M_PER):
                x_t = xpool.tile([128, DT * SN], F16)
                b1_t = bpool.tile([128, JT], F32, tag="b1")
                b2_t = bpool.tile([128, OT], F32, tag="b2")
                w1_first = w1pool.tile([128, CH * DT * 128], F16, tag="w1")
                if m == 0:
                    # Critical head path.  The scalar queue's prologue ends
                    # ~1 us before sync's, so it carries the first k-tile of
                    # w1 and x in small pieces (each k-tile's matmul starts
                    # as soon as its slice lands); sync carries the rest of
                    # the first w1 chunk in parallel on its own ring.
                    nc.scalar.dma_start(w1_first[:, 0:128], w1p[m, 0, :, 0:128])
                    nc.scalar.dma_start(x_t[:, 0:SN], xp[m, :, 0:SN])
                    nc.scalar.dma_start(x_t[:, SN : 2 * SN],
                                        xp[m, :, SN : 2 * SN])
                    nc.scalar.dma_start(x_t[:, 2 * SN : 3 * SN],
                                        xp[m, :, 2 * SN : 3 * SN])
                    nc.scalar.dma_start(x_t[:, 3 * SN : 5 * SN],
                                        xp[m, :, 3 * SN : 5 * SN])
                    nc.scalar.dma_start(x_t[:, 5 * SN :], xp[m, :, 5 * SN :])
                    nc.sync.dma_start(w1_first[:, 128 : DT * 128],
                                      w1p[m, 0, :, 128 : DT * 128])
                    nc.sync.dma_start(w1_first[:, DT * 128 :],
                                      w1p[m, 0, :, DT * 128 :])
                    nc.gpsimd.dma_start(b1_t[:], b1p[m])
                    nc.gpsimd.dma_start(b2_t[:], b2p[m])
                else:
                    nc.sync.dma_start(x_t[:], xp[m])
                    nc.sync.dma_start(w1_first[:], w1p[m, 0])
                    nc.gpsimd.dma_start(b1_t[:], b1p[m])
                    nc.gpsimd.dma_start(b2_t[:], b2p[m])

                h_t = hpool.tile([128, JT * SN], F16)
                for ch in range(NCH):
                    if ch == 0:
                        w1_t = w1_first
                    else:
                        w1_t = w1pool.tile([128, CH * DT * 128], F16, tag="w1")
                        nc.sync.dma_start(w1_t[:], w1p[m, ch])
                    for ji in range(CH):
                        jt = ch * CH + ji
                        ps = ps1pool.tile([128, SN], F32)
                        for k in range(DT):
                            nc.tensor.matmul(
                                ps[:],
                                w1_t[:, (ji * DT + k) * 128
                                     : (ji * DT + k + 1) * 128],
                                x_t[:, k * SN : (k + 1) * SN],
                                start=(k == 0),
                                stop=(k == DT - 1),
                            )
                        nc.scalar.activation(
                            h_t[:, jt * SN : (jt + 1) * SN],
                            ps[:],
                            relu,
                            bias=b1_t[:, jt : jt + 1],
                        )

                for ot in range(OT):
                    w2_t = w2pool.tile([128, JT * 128], F16)
                    nc.gpsimd.dma_start(w2_t[:], w2p[m, ot])
                    if m == M_PER - 1 and ot == OT - 1:
                        # Final output tile: two 256-wide halves so the first
                        # half's bias-add + store overlaps the second half's
                        # matmuls instead of serializing after the last one.
                        for half in range(2):
                            lo = half * (SN // 2)
                            hi = lo + SN // 2
                            ps2 = ps2pool.tile([128, SN // 2], F32, tag="ps2")
                            for k in range(JT):
                                nc.tensor.matmul(
                                    ps2[:],
                                    w2_t[:, k * 128 : (k + 1) * 128],
                                    h_t[:, k * SN + lo : k * SN + hi],
                                    start=(k == 0),
                                    stop=(k == JT - 1),
                                )
                            y_t = ypool.tile([128, SN // 2], F16, tag="y_t")
                            nc.scalar.activation(
                                y_t[:], ps2[:], ident, bias=b2_t[:, ot : ot + 1]
                            )
                            nc.scalar.dma_start(ytp[m, ot, :, lo:hi], y_t[:])
                        continue
                    ps2 = ps2pool.tile([128, SN], F32, tag="ps2")
                    for k in range(JT):
                        nc.tensor.matmul(
                            ps2[:],
                            w2_t[:, k * 128 : (k + 1) * 128],
                            h_t[:, k * SN : (k + 1) * SN],
                            start=(k == 0),
                            stop=(k == JT - 1),
                        )
                    y_t = ypool.tile([128, SN], F16, tag="y_t")
                    nc.scalar.activation(
                        y_t[:], ps2[:], ident, bias=b2_t[:, ot : ot + 1]
                    )
                    nc.scalar.dma_start(ytp[m, ot], y_t[:])
    nc.compile()
    return nc


def _pack_core(x_flat, ensemble_weights, members):
    """Pack one core's members into the DMA-friendly device layouts."""
    n = len(members)
    xp = np.empty((n, 128, DT * SN), dtype=NP_F16)
    w1p = np.empty((n, NCH, 128, CH * DT * 128), dtype=NP_F16)
    w2p = np.empty((n, OT, 128, JT * 128), dtype=NP_F16)
    b1p = np.empty((n, 128, JT), dtype=np.float32)
    b2p = np.empty((n, 128, OT), dtype=np.float32)
    for i, mem in enumerate(members):
        x = x_flat[mem].reshape(S, DIN)
        o = 0
        w1 = ensemble_weights[mem, o : o + N_W1].reshape(H, DIN); o += N_W1
        b1 = ensemble_weights[mem, o : o + N_B1]; o += N_B1
        w2 = ensemble_weights[mem, o : o + N_W2].reshape(DOUT, H); o += N_W2
        b2 = ensemble_weights[mem, o : o + N_B2]
        # xp[p, dt*S + s] = x[s, dt*128+p]
        xp[i] = (
            x.reshape(S, DT, 128).transpose(2, 1, 0).reshape(128, DT * SN)
        ).astype(NP_F16)
        # w1p[ch, p, (ji*DT + dt)*128 + jj] = w1[(ch*CH+ji)*128 + jj, dt*128 + p]
        w1p[i] = (
            w1.reshape(NCH, CH, 128, DT, 128)
            .transpose(0, 4, 1, 3, 2)
            .reshape(NCH, 128, CH * DT * 128)
        ).astype(NP_F16)
        # w2p[ot, p, jt*128+oo] = w2[ot*128+oo, jt*128+p]
        w2p[i] = (
            w2.reshape(OT, 128, JT, 128)
            .transpose(0, 3, 2, 1)
            .reshape(OT, 128, JT * 128)
        ).astype(NP_F16)
        b1p[i] = b1.reshape(JT, 128).T.astype(np.float32)
        b2p[i] = b2.reshape(OT, 128).T.astype(np.float32)
    return {"xp": xp, "w1p": w1p, "w2p": w2p, "b1p": b1p, "b2p": b2p}


def kernel(x_flat: np.ndarray, ensemble_weights: np.ndarray) -> np.ndarray:
    x_flat = np.asarray(x_flat, dtype=np.float32)
    ensemble_weights = np.asarray(ensemble_weights, dtype=np.float32)

    if "nc" not in _cache:
        _cache["nc"] = _build_nc()
    nc = _cache["nc"]

    in_maps = [
        _pack_core(x_flat, ensemble_weights,
                   list(range(c * M_PER, (c + 1) * M_PER)))
        for c in range(N_CORES)
    ]

    trace = bool(int(os.environ.get("KERNEL_TRACE", "0")))
    if trace:
        _install_ntff_shim()
    res = run_bass_kernel_spmd(nc, in_maps, core_ids=list(range(N_CORES)),
                               trace=trace)
    if trace:
        _cache["exec_time_ns"] = res.exec_time_ns

    out = np.empty((B, S * DOUT), dtype=np.float32)
    for c in range(N_CORES):
        ytp = res.results[c]["ytp"]  # (M_PER, OT, 128, SN) fp16
        for i in range(M_PER):
            mem = c * M_PER + i
            # y[s, ot*128+p] = ytp[i, ot, p, s]
            out[mem] = (
                ytp[i].astype(np.float32).transpose(2, 0, 1).reshape(S * DOUT)
            )
    return out


# revision 10
# speedup vs baseline: 1.0526x; 1.0526x over previous
"""Trainium2 Bass kernel for nn_EnsembleAdaptor: batched per-member MLP.

Per ensemble member (32 total): y = relu(x @ w1.T + b1) @ w2.T + b2
with x (512, 1024), w1 (4096, 1024), b1 (4096), w2 (1024, 4096), b2 (1024).

Sharding: pure data parallel over members — 4 members per core across 8 cores.

Device algorithm per member (fp16 operands, fp32 PSUM accumulation):
  layer 1 computes hT (H on partitions): for each j-tile (32), accumulate
    8 k-tiles of  psum[j,s] += w1T_tile.T @ xT_tile,  then ScalarE
    relu(psum + b1) -> hT sbuf tile (fp16).
  layer 2 computes yT (DOUT on partitions): for each o-tile (8), accumulate
    32 k-tiles of  psum[o,s] += w2T_tile.T @ hT_tile,  then ScalarE
    identity(psum + b2) -> fp16 sbuf -> DMA out as yT.

Schedule notes (from NTFF profile analysis):
  - The PE stream floor is 2048 matmuls x 216 ns; everything else must hide
    under it.  Only the sync (SP) queue's hardware DGE path delivers bulk
    data promptly — scalar/gpsimd-issued DMAs ramp far too slowly for
    critical tiles — so all weight/activation traffic rides sync.
  - w1 moves in 1 MB 4-j-tile chunks (fat descriptors ramp the DMA rings
    much faster than 256 KB per-j-tile ones and stop mid-L1 w1 starvation).
  - A memset'd dummy tile feeds a few warm-up matmuls at the very start so
    the PE's HAM activity monitor reaches the 2.4 GHz state while the first
    real tiles are still in flight (cold matmuls run at 1.2 GHz).
  - y is stored fp16: halves output bytes; adds ~1e-4 relative error
    against a 2e-2 gate.
"""

import contextlib
import ctypes
import os
import sys
import types

import numpy as np

import concourse.bass as bass
import concourse.tile as tile
from concourse import bacc, mybir
from concourse.bass_utils import run_bass_kernel_spmd


def _install_ntff_shim():
    """Provide antenv.axon_hooks + the ctypes NTFF profile hook when the
    image's antenv lacks them, so trace=True works under axon. Safe no-op
    on failure."""
    try:
        import antenv.axon_hooks  # noqa: F401
        return
    except ImportError:
        pass
    try:
        mod = types.ModuleType("antenv.axon_hooks")
        _state = {"hook": None}
        mod.set_axon_ntff_profile_hook = lambda h: _state.__setitem__("hook", h)
        mod.get_axon_ntff_profile_hook = lambda: _state["hook"]
        sys.modules["antenv.axon_hooks"] = mod
        import antenv
        antenv.axon_hooks = mod

        so_path = "/opt/axon/libaxon_pjrt.so"
        if not os.path.exists(so_path):
            return
        lib = ctypes.CDLL(so_path)
        if not hasattr(lib, "axon_start_nrt_profile"):
            return
        lib.axon_start_nrt_profile.argtypes = [
            ctypes.POINTER(ctypes.c_int64),
            ctypes.c_size_t,
        ]
        lib.axon_start_nrt_profile.restype = ctypes.c_int64
        lib.axon_stop_nrt_profile.argtypes = [ctypes.c_char_p]
        lib.axon_stop_nrt_profile.restype = ctypes.c_int64

        @contextlib.contextmanager
        def _hook(output_dir, device_ids):
            import jax
            jax.devices()
            if device_ids:
                ids = (ctypes.c_int64 * len(device_ids))(*device_ids)
                rc = lib.axon_start_nrt_profile(ids, len(device_ids))
            else:
                rc = lib.axon_start_nrt_profile(None, 0)
            if rc != 0:
                raise RuntimeError(f"axon_start_nrt_profile rc={rc}")
            try:
                yield
            finally:
                n = lib.axon_stop_nrt_profile(str(output_dir).encode())
                print(f"profile: {n} file(s) written to {output_dir}",
                      file=sys.stderr)

        mod.set_axon_ntff_profile_hook(_hook)
    except Exception:
        pass

B, S, DIN, H, DOUT = 32, 512, 1024, 4096, 1024
N_W1 = H * DIN
N_B1 = H
N_W2 = DOUT * H
N_B2 = DOUT

N_CORES = 8
M_PER = B // N_CORES  # members per core

DT = DIN // 128   # 8  k-tiles for layer 1
JT = H // 128     # 32 j-tiles (layer-1 outputs / layer-2 k-tiles)
OT = DOUT // 128  # 8  o-tiles for layer 2
SN = S            # 512 moving free dim

CH = 4            # j-tiles per w1 chunk
NCH = JT // CH    # 8 chunks per member

N_WARMUP = 8      # dummy matmuls to lift HAM to 2.4 GHz during the DMA head

F16 = mybir.dt.float16
F32 = mybir.dt.float32
NP_F16 = np.float16

_cache = {}


def _build_nc():
    nc = bacc.Bacc("TRN2", target_bir_lowering=False, enable_partition_id=False)
    xp = nc.dram_tensor("xp", [M_PER, 128, DT * SN], F16, kind="ExternalInput")
    w1p = nc.dram_tensor("w1p", [M_PER, NCH, 128, CH * DT * 128], F16,
                         kind="ExternalInput")
    w2p = nc.dram_tensor("w2p", [M_PER, OT, 128, JT * 128], F16,
                         kind="ExternalInput")
    b1p = nc.dram_tensor("b1p", [M_PER, 128, JT], F32, kind="ExternalInput")
    b2p = nc.dram_tensor("b2p", [M_PER, 128, OT], F32, kind="ExternalInput")
    ytp = nc.dram_tensor("ytp", [M_PER, OT, 128, SN], F16, kind="ExternalOutput")

    relu = mybir.ActivationFunctionType.Relu
    ident = mybir.ActivationFunctionType.Identity

    with tile.TileContext(nc) as tc:
        with (
            tc.tile_pool(name="xpool", bufs=2) as xpool,
            tc.tile_pool(name="w1pool", bufs=6) as w1pool,
            tc.tile_pool(name="w2pool", bufs=8) as w2pool,
            tc.tile_pool(name="bpool", bufs=2) as bpool,
            tc.tile_pool(name="hpool", bufs=1) as hpool,
            tc.tile_pool(name="ypool", bufs=4) as ypool,
            tc.tile_pool(name="dpool", bufs=1) as dpool,
            tc.tile_pool(name="ps1", bufs=4, space="PSUM") as ps1pool,
            tc.tile_pool(name="ps2", bufs=3, space="PSUM") as ps2pool,
        ):
            # PE warm-up: a few matmuls on a memset tile so the HAM clock
            # gate reaches 8/8 (2.4 GHz) while the first real DMAs land.
            dummy_t = dpool.tile([128, SN], F16)
            nc.vector.memset(dummy_t[:], 0.0)
            ps_d = ps2pool.tile([128, SN], F32, tag="dummy", bufs=1)
            for _ in range(N_WARMUP):
                nc.tensor.matmul(ps_d[:], dummy_t[:, 0:128], dummy_t[:],
                                 start=True, stop=True)

            for m in range(M_PER):
                x_t = xpool.tile([128, DT * SN], F16)
                b1_t = bpool.tile([128, JT], F32, tag="b1")
                b2_t = bpool.tile([128, OT], F32, tag="b2")
                w1_first = w1pool.tile([128, CH * DT * 128], F16, tag="w1")
                if m == 0:
                    # Critical head path: small descriptors so the first
                    # k-tile matmuls start as soon as their slices land.
                    nc.sync.dma_start(w1_first[:, 0:128], w1p[m, 0, :, 0:128])
                    nc.sync.dma_start(x_t[:, 0:SN], xp[m, :, 0:SN])
                    nc.sync.dma_start(w1_first[:, 128 : DT * 128],
                                      w1p[m, 0, :, 128 : DT * 128])
                    nc.sync.dma_start(x_t[:, SN : 2 * SN],
                                      xp[m, :, SN : 2 * SN])
                    nc.sync.dma_start(x_t[:, 2 * SN : 5 * SN],
                                      xp[m, :, 2 * SN : 5 * SN])
                    nc.sync.dma_start(x_t[:, 5 * SN :], xp[m, :, 5 * SN :])
                    nc.sync.dma_start(w1_first[:, DT * 128 :],
                                      w1p[m, 0, :, DT * 128 :])
                else:
                    nc.sync.dma_start(x_t[:], xp[m])
                    nc.sync.dma_start(w1_first[:], w1p[m, 0])
                nc.gpsimd.dma_start(b1_t[:], b1p[m])
                nc.gpsimd.dma_start(b2_t[:], b2p[m])

                h_t = hpool.tile([128, JT * SN], F16)
                for ch in range(NCH):
                    if ch == 0:
                        w1_t = w1_first
                    else:
                        w1_t = w1pool.tile([128, CH * DT * 128], F16, tag="w1")
                        nc.sync.dma_start(w1_t[:], w1p[m, ch])
                    for ji in range(CH):
                        jt = ch * CH + ji
                        ps = ps1pool.tile([128, SN], F32)
                        for k in range(DT):
                            nc.tensor.matmul(
                                ps[:],
                                w1_t[:, (ji * DT + k) * 128
                                     : (ji * DT + k + 1) * 128],
                                x_t[:, k * SN : (k + 1) * SN],
                                start=(k == 0),
                                stop=(k == DT - 1),
                            )
                        nc.scalar.activation(
                            h_t[:, jt * SN : (jt + 1) * SN],
                            ps[:],
                            relu,
                            bias=b1_t[:, jt : jt + 1],
                        )

                for ot in range(OT):
                    w2_t = w2pool.tile([128, JT * 128], F16)
                    nc.sync.dma_start(w2_t[:], w2p[m, ot])
                    if m == M_PER - 1 and ot == OT - 1:
                        # Final output tile: two 256-wide halves so the first
                        # half's bias-add + store overlaps the second half's
                        # matmuls instead of serializing after the last one.
                        for half in range(2):
                            lo = half * (SN // 2)
                            hi = lo + SN // 2
                            ps2 = ps2pool.tile([128, SN // 2], F32, tag="ps2")
                            for k in range(JT):
                                nc.tensor.matmul(
                                    ps2[:],
                                    w2_t[:, k * 128 : (k + 1) * 128],
                                    h_t[:, k * SN + lo : k * SN + hi],
                                    start=(k == 0),
                                    stop=(k == JT - 1),
                                )
                            y_t = ypool.tile([128, SN // 2], F16, tag="y_t")
                            nc.scalar.activation(
                                y_t[:], ps2[:], ident, bias=b2_t[:, ot : ot + 1]
                            )
                            nc.sync.dma_start(ytp[m, ot, :, lo:hi], y_t[:])
                        continue
                    ps2 = ps2pool.tile([128, SN], F32, tag="ps2")
                    for k in range(JT):
                        nc.tensor.matmul(
                            ps2[:],
                            w2_t[:, k * 128 : (k + 1) * 128],
                            h_t[:, k * SN : (k + 1) * SN],
                            start=(k == 0),
                            stop=(k == JT - 1),
                        )
                    y_t = ypool.tile([128, SN], F16, tag="y_t")
                    nc.scalar.activation(
                        y_t[:], ps2[:], ident, bias=b2_t[:, ot : ot + 1]
                    )
                    nc.sync.dma_start(ytp[m, ot], y_t[:])
    nc.compile()
    return nc


def _pack_core(x_flat, ensemble_weights, members):
    """Pack one core's members into the DMA-friendly device layouts."""
    n = len(members)
    xp = np.empty((n, 128, DT * SN), dtype=NP_F16)
    w1p = np.empty((n, NCH, 128, CH * DT * 128), dtype=NP_F16)
    w2p = np.empty((n, OT, 128, JT * 128), dtype=NP_F16)
    b1p = np.empty((n, 128, JT), dtype=np.float32)
    b2p = np.empty((n, 128, OT), dtype=np.float32)
    for i, mem in enumerate(members):
        x = x_flat[mem].reshape(S, DIN)
        o = 0
        w1 = ensemble_weights[mem, o : o + N_W1].reshape(H, DIN); o += N_W1
        b1 = ensemble_weights[mem, o : o + N_B1]; o += N_B1
        w2 = ensemble_weights[mem, o : o + N_W2].reshape(DOUT, H); o += N_W2
        b2 = ensemble_weights[mem, o : o + N_B2]
        # xp[p, dt*S + s] = x[s, dt*128+p]
        xp[i] = (
            x.reshape(S, DT, 128).transpose(2, 1, 0).reshape(128, DT * SN)
        ).astype(NP_F16)
        # w1p[ch, p, (ji*DT + dt)*128 + jj] = w1[(ch*CH+ji)*128 + jj, dt*128 + p]
        w1p[i] = (
            w1.reshape(NCH, CH, 128, DT, 128)
            .transpose(0, 4, 1, 3, 2)
            .reshape(NCH, 128, CH * DT * 128)
        ).astype(NP_F16)
        # w2p[ot, p, jt*128+oo] = w2[ot*128+oo, jt*128+p]
        w2p[i] = (
            w2.reshape(OT, 128, JT, 128)
            .transpose(0, 3, 2, 1)
            .reshape(OT, 128, JT * 128)
        ).astype(NP_F16)
        b1p[i] = b1.reshape(JT, 128).T.astype(np.float32)
        b2p[i] = b2.reshape(OT, 128).T.astype(np.float32)
    return {"xp": xp, "w1p": w1p, "w2p": w2p, "b1p": b1p, "b2p": b2p}


def kernel(x_flat: np.ndarray, ensemble_weights: np.ndarray) -> np.ndarray:
    x_flat = np.asarray(x_flat, dtype=np.float32)
    ensemble_weights = np.asarray(ensemble_weights, dtype=np.float32)

    if "nc" not in _cache:
        _cache["nc"] = _build_nc()
    nc = _cache["nc"]

    in_maps = [
        _pack_core(x_flat, ensemble_weights,
                   list(range(c * M_PER, (c + 1) * M_PER)))
        for c in range(N_CORES)
    ]

    trace = bool(int(os.environ.get("KERNEL_TRACE", "0")))
    if trace:
        _install_ntff_shim()
    res = run_bass_kernel_spmd(nc, in_maps, core_ids=list(range(N_CORES)),
                               trace=trace)
    if trace:
        _cache["exec_time_ns"] = res.exec_time_ns

    out = np.empty((B, S * DOUT), dtype=np.float32)
    for c in range(N_CORES):
        ytp = res.results[c]["ytp"]  # (M_PER, OT, 128, SN) fp16
        for i in range(M_PER):
            mem = c * M_PER + i
            # y[s, ot*128+p] = ytp[i, ot, p, s]
            out[mem] = (
                ytp[i].astype(np.float32).transpose(2, 0, 1).reshape(S * DOUT)
            )
    return out


# revision 16
# speedup vs baseline: 1.0607x; 1.0078x over previous
"""Trainium2 Bass kernel for nn_EnsembleAdaptor: batched per-member MLP.

Per ensemble member (32 total): y = relu(x @ w1.T + b1) @ w2.T + b2
with x (512, 1024), w1 (4096, 1024), b1 (4096), w2 (1024, 4096), b2 (1024).

Sharding: pure data parallel over members — 4 members per core across 8 cores.

Device algorithm per member (fp16 operands, fp32 PSUM accumulation):
  layer 1 computes hT (H on partitions): for each j-tile (32), accumulate
    8 k-tiles of  psum[j,s] += w1T_tile.T @ xT_tile,  then ScalarE
    relu(psum + b1) -> hT sbuf tile (fp16).
  layer 2 computes yT (DOUT on partitions): for each o-tile (8), accumulate
    32 k-tiles of  psum[o,s] += w2T_tile.T @ hT_tile,  then ScalarE
    identity(psum + b2) -> fp16 sbuf -> DMA out as yT.

Schedule notes (from NTFF profile analysis):
  - The PE stream floor is 2048 matmuls x 216 ns; everything else must hide
    under it.  Only the sync (SP) queue's hardware DGE path delivers bulk
    data promptly — scalar/gpsimd-issued DMAs ramp far too slowly for
    critical tiles — so all weight/activation traffic rides sync.
  - w1 moves in 1 MB 4-j-tile chunks (fat descriptors ramp the DMA rings
    much faster than 256 KB per-j-tile ones and stop mid-L1 w1 starvation).
  - A memset'd dummy tile feeds a few warm-up matmuls at the very start so
    the PE's HAM activity monitor reaches the 2.4 GHz state while the first
    real tiles are still in flight (cold matmuls run at 1.2 GHz).
  - y is stored fp16: halves output bytes; adds ~1e-4 relative error
    against a 2e-2 gate.
"""

import contextlib
import ctypes
import os
import sys
import types

import numpy as np

import concourse.bass as bass
import concourse.tile as tile
from concourse import bacc, mybir
from concourse.bass_utils import run_bass_kernel_spmd


def _install_ntff_shim():
    """Provide antenv.axon_hooks + the ctypes NTFF profile hook when the
    image's antenv lacks them, so trace=True works under axon. Safe no-op
    on failure."""
    try:
        import antenv.axon_hooks  # noqa: F401
        return
    except ImportError:
        pass
    try:
        mod = types.ModuleType("antenv.axon_hooks")
        _state = {"hook": None}
        mod.set_axon_ntff_profile_hook = lambda h: _state.__setitem__("hook", h)
        mod.get_axon_ntff_profile_hook = lambda: _state["hook"]
        sys.modules["antenv.axon_hooks"] = mod
        import antenv
        antenv.axon_hooks = mod

        so_path = "/opt/axon/libaxon_pjrt.so"
        if not os.path.exists(so_path):
            return
        lib = ctypes.CDLL(so_path)
        if not hasattr(lib, "axon_start_nrt_profile"):
            return
        lib.axon_start_nrt_profile.argtypes = [
            ctypes.POINTER(ctypes.c_int64),
            ctypes.c_size_t,
        ]
        lib.axon_start_nrt_profile.restype = ctypes.c_int64
        lib.axon_stop_nrt_profile.argtypes = [ctypes.c_char_p]
        lib.axon_stop_nrt_profile.restype = ctypes.c_int64

        @contextlib.contextmanager
        def _hook(output_dir, device_ids):
            import jax
            jax.devices()
            if device_ids:
                ids = (ctypes.c_int64 * len(device_ids))(*device_ids)
                rc = lib.axon_start_nrt_profile(ids, len(device_ids))
            else:
                rc = lib.axon_start_nrt_profile(None, 0)
            if rc != 0:
                raise RuntimeError(f"axon_start_nrt_profile rc={rc}")
            try:
                yield
            finally:
                n = lib.axon_stop_nrt_profile(str(output_dir).encode())
                print(f"profile: {n} file(s) written to {output_dir}",
                      file=sys.stderr)

        mod.set_axon_ntff_profile_hook(_hook)
    except Exception:
        pass

B, S, DIN, H, DOUT = 32, 512, 1024, 4096, 1024
N_W1 = H * DIN
N_B1 = H
N_W2 = DOUT * H
N_B2 = DOUT

N_CORES = 8
M_PER = B // N_CORES  # members per core

DT = DIN // 128   # 8  k-tiles for layer 1
JT = H // 128     # 32 j-tiles (layer-1 outputs / layer-2 k-tiles)
OT = DOUT // 128  # 8  o-tiles for layer 2
SN = S            # 512 moving free dim

CH = 4            # j-tiles per w1 chunk
NCH = JT // CH    # 8 chunks per member

N_WARMUP = 12     # dummy matmuls to lift HAM to 2.4 GHz during the DMA head

F16 = mybir.dt.float16
F32 = mybir.dt.float32
NP_F16 = np.float16

_cache = {}


def _build_nc():
    nc = bacc.Bacc("TRN2", target_bir_lowering=False, enable_partition_id=False)
    xp = nc.dram_tensor("xp", [M_PER, 128, DT * SN], F16, kind="ExternalInput")
    w1p = nc.dram_tensor("w1p", [M_PER, NCH, 128, CH * DT * 128], F16,
                         kind="ExternalInput")
    w2p = nc.dram_tensor("w2p", [M_PER, OT, 128, JT * 128], F16,
                         kind="ExternalInput")
    b1p = nc.dram_tensor("b1p", [M_PER, 128, JT], F32, kind="ExternalInput")
    b2p = nc.dram_tensor("b2p", [M_PER, 128, OT], F32, kind="ExternalInput")
    ytp = nc.dram_tensor("ytp", [M_PER, OT, 128, SN], F16, kind="ExternalOutput")

    relu = mybir.ActivationFunctionType.Relu
    ident = mybir.ActivationFunctionType.Identity

    with tile.TileContext(nc) as tc:
        with (
            tc.tile_pool(name="xpool", bufs=2) as xpool,
            tc.tile_pool(name="w1pool", bufs=6) as w1pool,
            tc.tile_pool(name="w2pool", bufs=8) as w2pool,
            tc.tile_pool(name="bpool", bufs=2) as bpool,
            tc.tile_pool(name="hpool", bufs=1) as hpool,
            tc.tile_pool(name="ypool", bufs=4) as ypool,
            tc.tile_pool(name="dpool", bufs=1) as dpool,
            tc.tile_pool(name="ps1", bufs=4, space="PSUM") as ps1pool,
            tc.tile_pool(name="ps2", bufs=3, space="PSUM") as ps2pool,
        ):
            # PE warm-up: a few matmuls on a memset tile so the HAM clock
            # gate reaches 8/8 (2.4 GHz) while the first real DMAs land.
            dummy_t = dpool.tile([128, SN], F16)
            nc.vector.memset(dummy_t[:], 0.0)
            ps_d = ps2pool.tile([128, SN], F32, tag="dummy", bufs=1)
            for _ in range(N_WARMUP):
                nc.tensor.matmul(ps_d[:], dummy_t[:, 0:128], dummy_t[:],
                                 start=True, stop=True)

            for m in range(M_PER):
                x_t = xpool.tile([128, DT * SN], F16)
                b1_t = bpool.tile([128, JT], F32, tag="b1")
                b2_t = bpool.tile([128, OT], F32, tag="b2")
                w1_first = w1pool.tile([128, CH * DT * 128], F16, tag="w1")
                if m == 0:
                    # Critical head path.  Delivery rate is set by the
                    # per-partition line length of each descriptor (1-2 KB
                    # lines move at ~150 GB/s, 4-8 KB at ~450 GB/s), so the
                    # head moves in four fat-line 512 KB pieces: w1 j-tiles
                    # 0-1, x k-tiles 0-3, x k-tiles 4-7, w1 j-tiles 2-3.
                    HW = CH * DT * 128 // 2
                    nc.sync.dma_start(w1_first[:, 0:HW], w1p[m, 0, :, 0:HW])
                    nc.sync.dma_start(x_t[:, 0 : 4 * SN], xp[m, :, 0 : 4 * SN])
                    nc.sync.dma_start(x_t[:, 4 * SN :], xp[m, :, 4 * SN :])
                    nc.sync.dma_start(w1_first[:, HW:], w1p[m, 0, :, HW:])
                else:
                    nc.sync.dma_start(x_t[:], xp[m])
                    nc.sync.dma_start(w1_first[:], w1p[m, 0])
                nc.gpsimd.dma_start(b1_t[:], b1p[m])
                nc.gpsimd.dma_start(b2_t[:], b2p[m])

                h_t = hpool.tile([128, JT * SN], F16)
                for ch in range(NCH):
                    if ch == 0:
                        w1_t = w1_first
                    else:
                        w1_t = w1pool.tile([128, CH * DT * 128], F16, tag="w1")
                        nc.sync.dma_start(w1_t[:], w1p[m, ch])
                    if m == 0 and ch == 0:
                        # Match the head DMA split: j-tiles 0-1 run k 0-3 as
                        # soon as the first two pieces land, finishing k 4-7
                        # when the x tail arrives; j-tiles 2-3 follow.
                        ps_ab = [ps1pool.tile([128, SN], F32, name=f"ps_h{i}",
                                              tag="ps")
                                 for i in range(2)]
                        for khalf in range(2):
                            for ji in range(2):
                                for k in range(khalf * 4, khalf * 4 + 4):
                                    nc.tensor.matmul(
                                        ps_ab[ji][:],
                                        w1_t[:, (ji * DT + k) * 128
                                             : (ji * DT + k + 1) * 128],
                                        x_t[:, k * SN : (k + 1) * SN],
                                        start=(k == 0),
                                        stop=(k == DT - 1),
                                    )
                        for ji in range(2):
                            nc.scalar.activation(
                                h_t[:, ji * SN : (ji + 1) * SN],
                                ps_ab[ji][:],
                                relu,
                                bias=b1_t[:, ji : ji + 1],
                            )
                        jis = range(2, CH)
                    else:
                        jis = range(CH)
                    for ji in jis:
                        jt = ch * CH + ji
                        ps = ps1pool.tile([128, SN], F32, tag="ps")
                        for k in range(DT):
                            nc.tensor.matmul(
                                ps[:],
                                w1_t[:, (ji * DT + k) * 128
                                     : (ji * DT + k + 1) * 128],
                                x_t[:, k * SN : (k + 1) * SN],
                                start=(k == 0),
                                stop=(k == DT - 1),
                            )
                        nc.scalar.activation(
                            h_t[:, jt * SN : (jt + 1) * SN],
                            ps[:],
                            relu,
                            bias=b1_t[:, jt : jt + 1],
                        )

                for ot in range(OT):
                    w2_t = w2pool.tile([128, JT * 128], F16)
                    nc.sync.dma_start(w2_t[:], w2p[m, ot])
                    if m == M_PER - 1 and ot == OT - 1:
                        # Final output tile: two 256-wide halves so the first
                        # half's bias-add + store overlaps the second half's
                        # matmuls instead of serializing after the last one.
                        for half in range(2):
                            lo = half * (SN // 2)
                            hi = lo + SN // 2
                            ps2 = ps2pool.tile([128, SN // 2], F32, tag="ps2")
                            for k in range(JT):
                                nc.tensor.matmul(
                                    ps2[:],
                                    w2_t[:, k * 128 : (k + 1) * 128],
                                    h_t[:, k * SN + lo : k * SN + hi],
                                    start=(k == 0),
                                    stop=(k == JT - 1),
                                )
                            y_t = ypool.tile([128, SN // 2], F16, tag="y_t")
                            nc.scalar.activation(
                                y_t[:], ps2[:], ident, bias=b2_t[:, ot : ot + 1]
                            )
                            nc.sync.dma_start(ytp[m, ot, :, lo:hi], y_t[:])
                        continue
                    ps2 = ps2pool.tile([128, SN], F32, tag="ps2")
                    for k in range(JT):
                        nc.tensor.matmul(
                            ps2[:],
                            w2_t[:, k * 128 : (k + 1) * 128],
                            h_t[:, k * SN : (k + 1) * SN],
                            start=(k == 0),
                            stop=(k == JT - 1),
                        )
                    y_t = ypool.tile([128, SN], F16, tag="y_t")
                    nc.scalar.activation(
                        y_t[:], ps2[:], ident, bias=b2_t[:, ot : ot + 1]
                    )
                    nc.sync.dma_start(ytp[m, ot], y_t[:])
    nc.compile()
    return nc


def _pack_core(x_flat, ensemble_weights, members):
    """Pack one core's members into the DMA-friendly device layouts."""
    n = len(members)
    xp = np.empty((n, 128, DT * SN), dtype=NP_F16)
    w1p = np.empty((n, NCH, 128, CH * DT * 128), dtype=NP_F16)
    w2p = np.empty((n, OT, 128, JT * 128), dtype=NP_F16)
    b1p = np.empty((n, 128, JT), dtype=np.float32)
    b2p = np.empty((n, 128, OT), dtype=np.float32)
    for i, mem in enumerate(members):
        x = x_flat[mem].reshape(S, DIN)
        o = 0
        w1 = ensemble_weights[mem, o : o + N_W1].reshape(H, DIN); o += N_W1
        b1 = ensemble_weights[mem, o : o + N_B1]; o += N_B1
        w2 = ensemble_weights[mem, o : o + N_W2].reshape(DOUT, H); o += N_W2
        b2 = ensemble_weights[mem, o : o + N_B2]
        # xp[p, dt*S + s] = x[s, dt*128+p]
        xp[i] = (
            x.reshape(S, DT, 128).transpose(2, 1, 0).reshape(128, DT * SN)
        ).astype(NP_F16)
        # w1p[ch, p, (ji*DT + dt)*128 + jj] = w1[(ch*CH+ji)*128 + jj, dt*128 + p]
        w1p[i] = (
            w1.reshape(NCH, CH, 128, DT, 128)
            .transpose(0, 4, 1, 3, 2)
            .reshape(NCH, 128, CH * DT * 128)
        ).astype(NP_F16)
        # w2p[ot, p, jt*128+oo] = w2[ot*128+oo, jt*128+p]
        w2p[i] = (
            w2.reshape(OT, 128, JT, 128)
            .transpose(0, 3, 2, 1)
            .reshape(OT, 128, JT * 128)
        ).astype(NP_F16)
        b1p[i] = b1.reshape(JT, 128).T.astype(np.float32)
        b2p[i] = b2.reshape(OT, 128).T.astype(np.float32)
    return {"xp": xp, "w1p": w1p, "w2p": w2p, "b1p": b1p, "b2p": b2p}


def kernel(x_flat: np.ndarray, ensemble_weights: np.ndarray) -> np.ndarray:
    x_flat = np.asarray(x_flat, dtype=np.float32)
    ensemble_weights = np.asarray(ensemble_weights, dtype=np.float32)

    if "nc" not in _cache:
        _cache["nc"] = _build_nc()
    nc = _cache["nc"]

    in_maps = [
        _pack_core(x_flat, ensemble_weights,
                   list(range(c * M_PER, (c + 1) * M_PER)))
        for c in range(N_CORES)
    ]

    trace = bool(int(os.environ.get("KERNEL_TRACE", "0")))
    if trace:
        _install_ntff_shim()
    res = run_bass_kernel_spmd(nc, in_maps, core_ids=list(range(N_CORES)),
                               trace=trace)
    if trace:
        _cache["exec_time_ns"] = res.exec_time_ns

    out = np.empty((B, S * DOUT), dtype=np.float32)
    for c in range(N_CORES):
        ytp = res.results[c]["ytp"]  # (M_PER, OT, 128, SN) fp16
        for i in range(M_PER):
            mem = c * M_PER + i
            # y[s, ot*128+p] = ytp[i, ot, p, s]
            out[mem] = (
                ytp[i].astype(np.float32).transpose(2, 0, 1).reshape(S * DOUT)
            )
    return out
